# revision 18
# baseline (speedup 1.0000x reference)
"""CenterLoss Trainium2 kernel (label-bucketed data-parallel over 8 cores).

loss = sum(clip(distmat * onehot(labels), 1e-12, 1e12)) / B,
distmat[i,c] = ||x_i - centers_c||^2. Only the (i, labels_i) entries survive
the mask; the B*(C-1) masked entries contribute exactly 1e-12 each (added
analytically on host). For this distribution d_i ~ 4096, so the clip never
binds and the sum decomposes exactly:

  sum_i d_i = sum_i ||x_i||^2 + sum_c n_c ||c_c||^2 - 2 sum_c <s_c, c_c>

with s = onehot(labels)^T @ x (computed per core over its local samples).

Sharding is the key bandwidth optimization: samples are SORTED BY LABEL on
host and split into 8 equal chunks of 1024. Each chunk spans a contiguous
label range (<= 128 classes for uniform labels), so each core only loads the
center rows its samples reference: ~0.8 MB instead of the full 6.15 MB
replica. Per-core HBM traffic drops from 14.6 MB to ~9.2 MB; x itself
(8.4 MB/core, read once) is the roofline term. Labels are relabeled to the
local window; the per-class counts for the n_c ||c_c||^2 term are a host
histogram (as in the torch reference's bincount) dotted with the
device-computed ||c_c||^2 column.

Device pipeline per core (fast path, 128-class window):
  - x streams as 7 full [128, 2048] tiles + tile 7 in column chunks
    (512x3 + 256x2) so the end-of-stream dependency chain is short.
  - ACT: Square+accum of tiles 0-5 and of the centers window (early);
    fp32->fp8 copies of tile 7's chunks at the tail (DVE is drain-bound).
  - DVE: fp32->fp8 copies of tiles 0-6, one-hot build, PSUM drains
    scalar_tensor_tensor (-2*S).C with accum, and a masked-diagonal stt.
  - PE: fp8 DoubleRow matmuls accumulate S = onehot^T x in 4 PSUM banks.
    ||x||^2 of tiles 6+7 comes from 16 extra "gram" matmuls x8^T @ x8
    accumulated in a 5th PSUM bank: its diagonal is sum x^2 over those 512
    samples per feature-residue; one eye-masked stt extracts it. This keeps
    all late-arriving-square work off ACT/DVE's critical tail.

Per-core output is a [128, OUTW] block of raw partial columns; host combine
(f64) sums them, adds <histogram, cn2> and B*(C-1)*1e-12, divides by B.
"""

from contextlib import ExitStack

import numpy as np

import concourse.bacc as bacc
import concourse.tile as tile
from concourse import mybir
from concourse.bass_utils import run_bass_kernel_spmd

N_CORES = 8
B = 8192
D = 2048
C = 751
BS = B // N_CORES  # samples per core
P = 128
NT = BS // P       # sample tiles per core (8)
KDR = NT // 2      # fp8 DoubleRow k-tiles (256 samples each)
NCH = 4            # 512-col feature chunks (one PSUM bank each)
CH = D // NCH      # 512
FP8 = mybir.dt.float8e4
CLIP_LO = 1e-12
MULT = mybir.AluOpType.mult

# Default geometry matches the reference's seed-0 data (max label span 99).
DEF_CR = 99
DEF_MT = 1

# Fast-path output columns: 0 gram diag (sum x^2, all tiles), 1-6 per-bank
# cross-term drains, 7 cn2 (||c||^2 per local class).
F_GRAM, F_STT0, F_CN, F_OUTW = 0, 1, 7, 8

# Column groups: one PSUM bank each; the tail groups are narrow so the
# final drain chain after the last DMA byte is short.
F_GROUPS = [(0, 512), (512, 1024), (1024, 1536), (1536, 1792), (1792, 1920), (1920, 2048)]

_NC_CACHE = {}


def build_nc(cr=DEF_CR, mt=DEF_MT):
    if mt == 1:
        return _build_fast(cr)
    return _build_generic(cr, mt)


def _build_fast(cr):
    """Optimized single-class-tile program (label span <= 128).

    x streams column-group-major: for each 512/256-col group, the 4
    DoubleRow k-pairs arrive back to back, their fp8 copies feed the S
    matmuls for that group's PSUM bank plus the gram matmuls, and the bank
    drains immediately -- so drains pipeline with the DMA stream instead of
    stacking up after it. Sum x^2 comes entirely from the gram bank's
    diagonal (fp8: ~1e-3 relative bias, far inside the 2e-2 gate).
    """
    nc = bacc.Bacc("TRN2", target_bir_lowering=False)
    x = nc.dram_tensor("x", [BS, D], mybir.dt.float32, kind="ExternalInput")
    labels = nc.dram_tensor("labels", [P, NT], mybir.dt.int32, kind="ExternalInput")
    centers = nc.dram_tensor("centers", [cr, D], mybir.dt.float32, kind="ExternalInput")
    out = nc.dram_tensor("partial", [P, F_OUTW], mybir.dt.float32, kind="ExternalOutput")

    x_r = x[:].rearrange("(t p) d -> p t d", p=P)  # x_r[p, t, :] = x[t*128+p]

    with tile.TileContext(nc) as tc, ExitStack() as ctx:
        xp = ctx.enter_context(tc.tile_pool(name="xp", bufs=6))
        sqp = ctx.enter_context(tc.tile_pool(name="sqp", bufs=1))
        stp = ctx.enter_context(tc.tile_pool(name="stp", bufs=2))
        perm = ctx.enter_context(tc.tile_pool(name="perm", bufs=1))
        psp = ctx.enter_context(tc.tile_pool(name="psp", bufs=1, space="PSUM"))

        # labels + centers ride the ACT HWDGE ring; x owns the SP ring
        lab = perm.tile([P, NT], mybir.dt.int32)
        nc.scalar.dma_start(out=lab[:], in_=labels[:])
        ct = perm.tile([P, D], mybir.dt.float32)
        if cr < P:
            # pad rows must be finite zeros; memset start must be 32-aligned
            nc.gpsimd.memset(ct[(cr // 32) * 32 :, :], 0.0)
        nc.scalar.dma_start(out=ct[:cr, :], in_=centers[:, :])

        iota_i = perm.tile([P, P], mybir.dt.int32)
        nc.gpsimd.iota(iota_i[:], pattern=[[1, P]], base=0, channel_multiplier=0)
        pidx_i = perm.tile([P, 1], mybir.dt.int32)
        nc.gpsimd.iota(pidx_i[:], pattern=[[0, 1]], base=0, channel_multiplier=1)
        iota_f = perm.tile([P, P], mybir.dt.float32)
        nc.vector.tensor_copy(out=iota_f[:], in_=iota_i[:])
        pidx_f = perm.tile([P, 1], mybir.dt.float32)
        nc.vector.tensor_copy(out=pidx_f[:], in_=pidx_i[:])
        lab_f = perm.tile([P, NT], mybir.dt.float32)
        nc.vector.tensor_copy(out=lab_f[:], in_=lab[:])
        # eye_f[p, j] = (j == p), mask for the gram-diagonal extraction
        eye_f = perm.tile([P, P], mybir.dt.float32)
        nc.vector.tensor_scalar(
            out=eye_f[:], in0=iota_f[:], scalar1=pidx_f[:],
            scalar2=None, op0=mybir.AluOpType.is_equal,
        )

        out_sb = perm.tile([P, F_OUTW], mybir.dt.float32)
        nc.vector.memset(out_sb[:], 0.0)

        # ||c||^2 early on ACT (its only compute)
        sqc = sqp.tile([P, D], mybir.dt.float32)
        nc.scalar.activation(
            out=sqc[:cr, :], in_=ct[:cr, :],
            func=mybir.ActivationFunctionType.Square,
            accum_out=out_sb[:cr, F_CN : F_CN + 1],
        )

        # fp8 DoubleRow-packed x and one-hot: [128, 2, *], j = sample tile 2k+j
        x8 = [perm.tile([P, 2, D], FP8, tag=f"x8_{k}", name=f"x8_{k}")
              for k in range(KDR)]
        oh8 = [perm.tile([P, 2, P], FP8, tag=f"oh8_{k}", name=f"oh8_{k}")
               for k in range(KDR)]
        for t in range(NT):
            k, j = divmod(t, 2)
            nc.vector.tensor_scalar(
                out=oh8[k][:, j, :], in0=iota_f[:], scalar1=lab_f[:, t : t + 1],
                scalar2=None, op0=mybir.AluOpType.is_equal,
            )

        # PSUM: 5 S banks (one per column group) + 1 gram bank
        ps = [psp.tile([P, CH], mybir.dt.float32, tag=f"ps{g}", name=f"ps{g}")
              for g in range(len(F_GROUPS))]
        gram = psp.tile([P, CH], mybir.dt.float32, tag="gram")

        last_g = len(F_GROUPS) - 1

        def drain(g):
            # accum (-2 * S_g) . C -> col on DVE
            lo, hi = F_GROUPS[g]
            so = stp.tile([P, CH], mybir.dt.float32, tag="so", name="so")
            nc.vector.scalar_tensor_tensor(
                out=so[:, : hi - lo], in0=ps[g][:, : hi - lo], scalar=-2.0,
                in1=ct[:, lo:hi], op0=MULT, op1=MULT,
                accum_out=out_sb[:, F_STT0 + g : F_STT0 + g + 1],
            )

        for g, (lo, hi) in enumerate(F_GROUPS):
            w = hi - lo
            for k in range(KDR):
                # one DMA per (k-pair, group): both DoubleRow rows arrive
                # together (uniform staging keeps the scheduler's DMA order)
                stg = xp.tile([P, 2, w], mybir.dt.float32, tag="stg")
                nc.sync.dma_start(out=stg[:], in_=x_r[:, 2 * k : 2 * k + 2, lo:hi])
                nc.vector.tensor_copy(out=x8[k][:, :, lo:hi], in_=stg[:])
                nc.tensor.matmul(
                    out=ps[g][:, :w], lhsT=oh8[k][:, :, :],
                    rhs=x8[k][:, :, lo:hi],
                    start=(k == 0), stop=(k == KDR - 1),
                    perf_mode=mybir.MatmulPerfMode.DoubleRow,
                )
                for a in range(lo // P, -(-hi // P)):
                    blk = slice(a * P, (a + 1) * P)
                    first = (g == 0 and k == 0 and a == 0)
                    last = (g == last_g and k == KDR - 1 and blk.stop >= hi)
                    nc.tensor.matmul(
                        out=gram[:, :P],
                        lhsT=x8[k][:, :, blk], rhs=x8[k][:, :, blk],
                        start=first, stop=last,
                        perf_mode=mybir.MatmulPerfMode.DoubleRow,
                        skip_group_check=True,
                    )
                if k == 2 and g >= 1:
                    # previous group's drain, deferred so it never blocks
                    # this group's copies on DVE's in-order queue
                    drain(g - 1)
        drain(last_g)

        # gram diagonal -> col 0 (one eye-masked stt)
        dso = stp.tile([P, P], mybir.dt.float32, tag="dso", name="dso")
        nc.vector.scalar_tensor_tensor(
            out=dso[:], in0=gram[:, :P], scalar=1.0, in1=eye_f[:],
            op0=MULT, op1=MULT, accum_out=out_sb[:, F_GRAM : F_GRAM + 1],
        )

        nc.sync.dma_start(out=out[:], in_=out_sb[:])
    nc.compile()
    return nc


# ---------------------------------------------------------------------------
# Generic fallback for pathological label distributions (span > 128 classes).
# Correctness-first: squares all on ACT, centers in mt class tiles.
G_FULL_T = NT - 2


def _gcols(mt):
    stt0 = G_FULL_T + 2 * NCH
    cn0 = stt0 + mt * NCH
    outw = -(-(cn0 + mt) // 4) * 4
    return stt0, cn0, outw


def _build_generic(cr, mt):
    M = P * mt
    STT0, CN0, OUTW = _gcols(mt)
    nc = bacc.Bacc("TRN2", target_bir_lowering=False)
    x = nc.dram_tensor("x", [BS, D], mybir.dt.float32, kind="ExternalInput")
    labels = nc.dram_tensor("labels", [P, NT], mybir.dt.int32, kind="ExternalInput")
    centers = nc.dram_tensor("centers", [cr, D], mybir.dt.float32, kind="ExternalInput")
    out = nc.dram_tensor("partial", [P, OUTW], mybir.dt.float32, kind="ExternalOutput")

    x_r = x[:].rearrange("(t p) d -> p t d", p=P)

    with tile.TileContext(nc) as tc, ExitStack() as ctx:
        xp = ctx.enter_context(tc.tile_pool(name="xp", bufs=3))
        sqp = ctx.enter_context(tc.tile_pool(name="sqp", bufs=2))
        csq = ctx.enter_context(tc.tile_pool(name="csq", bufs=2))
        stp = ctx.enter_context(tc.tile_pool(name="stp", bufs=2))
        perm = ctx.enter_context(tc.tile_pool(name="perm", bufs=1))
        psp = ctx.enter_context(tc.tile_pool(name="psp", bufs=1, space="PSUM"))

        lab = perm.tile([P, NT], mybir.dt.int32)
        nc.scalar.dma_start(out=lab[:], in_=labels[:])

        cts = []
        for m in range(mt):
            rows = min(cr - m * P, P)
            ct = perm.tile([P, D], mybir.dt.float32, tag=f"ct{m}")
            if rows < P:
                nc.gpsimd.memset(ct[(rows // 32) * 32 :, :], 0.0)
            nc.scalar.dma_start(out=ct[:rows, :], in_=centers[m * P : m * P + rows, :])
            cts.append(ct)

        iota_i = perm.tile([P, M], mybir.dt.int32)
        nc.gpsimd.iota(iota_i[:], pattern=[[1, M]], base=0, channel_multiplier=0)
        iota_f = perm.tile([P, M], mybir.dt.float32)
        nc.vector.tensor_copy(out=iota_f[:], in_=iota_i[:])
        lab_f = perm.tile([P, NT], mybir.dt.float32)
        nc.vector.tensor_copy(out=lab_f[:], in_=lab[:])

        out_sb = perm.tile([P, OUTW], mybir.dt.float32)
        nc.vector.memset(out_sb[:], 0.0)

        x8 = [perm.tile([P, 2, D], FP8, tag=f"x8_{k}", name=f"x8_{k}")
              for k in range(KDR)]
        oh8 = [perm.tile([P, 2, M], FP8, tag=f"oh8_{k}", name=f"oh8_{k}")
               for k in range(KDR)]
        for t in range(NT):
            k, j = divmod(t, 2)
            nc.vector.tensor_scalar(
                out=oh8[k][:, j, :], in0=iota_f[:], scalar1=lab_f[:, t : t + 1],
                scalar2=None, op0=mybir.AluOpType.is_equal,
            )

        for t in range(G_FULL_T):
            k, j = divmod(t, 2)
            xt = xp.tile([P, D], mybir.dt.float32, tag="xt")
            nc.sync.dma_start(out=xt[:], in_=x_r[:, t, :])
            sq = sqp.tile([P, D], mybir.dt.float32, tag="sq")
            nc.scalar.activation(
                out=sq[:], in_=xt[:], func=mybir.ActivationFunctionType.Square,
                accum_out=out_sb[:, t : t + 1],
            )
            if t == 0:
                for m in range(mt):
                    rows = min(cr - m * P, P)
                    sqc = sqp.tile([P, D], mybir.dt.float32, tag="sq")
                    nc.scalar.activation(
                        out=sqc[:rows, :], in_=cts[m][:rows, :],
                        func=mybir.ActivationFunctionType.Square,
                        accum_out=out_sb[:rows, CN0 + m : CN0 + m + 1],
                    )
            nc.vector.tensor_copy(out=x8[k][:, j, :], in_=xt[:])

        xtail = [perm.tile([P, D], mybir.dt.float32, tag=f"xt{t}", name=f"xtail{t}")
                 for t in (0, 1)]
        for i, t in enumerate((G_FULL_T, G_FULL_T + 1)):
            for n in range(NCH):
                sl = slice(n * CH, (n + 1) * CH)
                nc.sync.dma_start(out=xtail[i][:, sl], in_=x_r[:, t, sl])

        for i in range(2):
            for n in range(NCH):
                sl = slice(n * CH, (n + 1) * CH)
                cq = csq.tile([P, CH], mybir.dt.float32, tag="cq")
                nc.scalar.activation(
                    out=cq[:], in_=xtail[i][:, sl],
                    func=mybir.ActivationFunctionType.Square,
                    accum_out=out_sb[:, G_FULL_T + i * NCH + n :
                                     G_FULL_T + i * NCH + n + 1],
                )

        for n in range(NCH):
            sl = slice(n * CH, (n + 1) * CH)
            nc.vector.tensor_copy(out=x8[KDR - 1][:, 0, sl], in_=xtail[0][:, sl])
            nc.vector.tensor_copy(out=x8[KDR - 1][:, 1, sl], in_=xtail[1][:, sl])
        for m in range(mt):
            pr = [psp.tile([P, CH], mybir.dt.float32, tag=f"ps{n}", name="pr")
                  for n in range(NCH)]
            for k in range(KDR):
                for n in range(NCH):
                    nc.tensor.matmul(
                        out=pr[n][:], lhsT=oh8[k][:, :, m * P : (m + 1) * P],
                        rhs=x8[k][:, :, n * CH : (n + 1) * CH],
                        start=(k == 0), stop=(k == KDR - 1),
                        perf_mode=mybir.MatmulPerfMode.DoubleRow,
                    )
            for n in range(NCH):
                so = stp.tile([P, CH], mybir.dt.float32, tag="so", name="so")
                nc.vector.scalar_tensor_tensor(
                    out=so[:], in0=pr[n][:], scalar=-2.0,
                    in1=cts[m][:, n * CH : (n + 1) * CH],
                    op0=MULT, op1=MULT,
                    accum_out=out_sb[:, STT0 + m * NCH + n : STT0 + m * NCH + n + 1],
                )

        nc.sync.dma_start(out=out[:], in_=out_sb[:])
    nc.compile()
    return nc


def _get_nc(cr, mt):
    key = (cr, mt)
    if key not in _NC_CACHE:
        _NC_CACHE[key] = build_nc(cr, mt)
    return _NC_CACHE[key]


def _shard(x, labels, centers):
    """Sort samples by label, split into 8 equal chunks, build per-core
    inputs with a local (relabeled) class window and its center rows."""
    order = np.argsort(labels, kind="stable")
    ls = labels[order]
    bases, spans = [], []
    for k in range(N_CORES):
        lo, hi = ls[k * BS], ls[(k + 1) * BS - 1]
        bases.append(int(lo))
        spans.append(int(hi - lo + 1))
    cr = max(spans)
    mt = -(-cr // P)
    in_maps, counts = [], []
    for k in range(N_CORES):
        idx = order[k * BS : (k + 1) * BS]
        xs = np.ascontiguousarray(x[idx])
        lloc = (ls[k * BS : (k + 1) * BS] - bases[k]).astype(np.int32)
        # lab[p, t] = lloc[t*P + p], matching the x tile layout
        lab = np.ascontiguousarray(lloc.reshape(NT, P).T)
        cw = np.zeros((cr, D), dtype=np.float32)
        cw[: spans[k]] = centers[bases[k] : bases[k] + spans[k]]
        in_maps.append({"x": xs, "labels": lab, "centers": cw})
        counts.append(np.bincount(lloc, minlength=mt * P).astype(np.float64))
    return in_maps, counts, cr, mt


def make_in_maps(x, labels, centers):
    return _shard(np.asarray(x), np.asarray(labels).astype(np.int64),
                  np.asarray(centers))[0]


def _combine(partials, counts, mt):
    if mt == 1:
        cn0, ncols = F_CN, F_CN
    else:
        _, cn0, _ = _gcols(mt)
        ncols = cn0
    total = 0.0
    for k, p in enumerate(partials):
        p64 = p.astype(np.float64)
        total += float(np.sum(p64[:, :ncols]))
        # n_c * ||c_c||^2: host label histogram x device-computed ||c||^2
        cn2 = p64[:, cn0 : cn0 + mt]  # class m*128 + partition
        total += float(np.sum(counts[k].reshape(mt, P).T * cn2))
    total += float(B) * float(C - 1) * CLIP_LO
    return np.array(total / B, dtype=np.float32)


def kernel(**inputs) -> np.ndarray:
    x = np.ascontiguousarray(np.asarray(inputs["x"], dtype=np.float32))
    labels = np.asarray(inputs["labels"]).astype(np.int64)
    centers = np.ascontiguousarray(np.asarray(inputs["centers"], dtype=np.float32))
    assert x.shape == (B, D) and labels.shape == (B,) and centers.shape == (C, D)

    in_maps, counts, cr, mt = _shard(x, labels, centers)
    nc = _get_nc(cr, mt)
    res = run_bass_kernel_spmd(nc, in_maps, core_ids=list(range(N_CORES)))
    return _combine([r["partial"] for r in res.results], counts, mt)


# revision 19
# speedup vs baseline: 1.0482x; 1.0482x over previous
"""CenterLoss Trainium2 kernel (label-bucketed data-parallel over 8 cores).

loss = sum(clip(distmat * onehot(labels), 1e-12, 1e12)) / B,
distmat[i,c] = ||x_i - centers_c||^2. Only the (i, labels_i) entries survive
the mask; the B*(C-1) masked entries contribute exactly 1e-12 each (added
analytically on host). For this distribution d_i ~ 4096, so the clip never
binds and the sum decomposes exactly:

  sum_i d_i = sum_i ||x_i||^2 + sum_c n_c ||c_c||^2 - 2 sum_c <s_c, c_c>

with s = onehot(labels)^T @ x (computed per core over its local samples).

Sharding is the key bandwidth optimization: samples are SORTED BY LABEL on
host and split into 8 equal chunks of 1024. Each chunk spans a contiguous
label range (<= 128 classes for uniform labels), so each core only loads the
center rows its samples reference: ~0.8 MB instead of the full 6.15 MB
replica. Per-core HBM traffic drops from 14.6 MB to ~9.2 MB; x itself
(8.4 MB/core, read once) is the roofline term. Labels are relabeled to the
local window; the per-class counts for the n_c ||c_c||^2 term are a host
histogram (as in the torch reference's bincount) dotted with the
device-computed ||c_c||^2 column.

Device pipeline per core (fast path, 128-class window):
  - x streams as 7 full [128, 2048] tiles + tile 7 in column chunks
    (512x3 + 256x2) so the end-of-stream dependency chain is short.
  - ACT: Square+accum of tiles 0-5 and of the centers window (early);
    fp32->fp8 copies of tile 7's chunks at the tail (DVE is drain-bound).
  - DVE: fp32->fp8 copies of tiles 0-6, one-hot build, PSUM drains
    scalar_tensor_tensor (-2*S).C with accum, and a masked-diagonal stt.
  - PE: fp8 DoubleRow matmuls accumulate S = onehot^T x in 4 PSUM banks.
    ||x||^2 of tiles 6+7 comes from 16 extra "gram" matmuls x8^T @ x8
    accumulated in a 5th PSUM bank: its diagonal is sum x^2 over those 512
    samples per feature-residue; one eye-masked stt extracts it. This keeps
    all late-arriving-square work off ACT/DVE's critical tail.

Per-core output is a [128, OUTW] block of raw partial columns; host combine
(f64) sums them, adds <histogram, cn2> and B*(C-1)*1e-12, divides by B.
"""

from contextlib import ExitStack

import numpy as np

import concourse.bacc as bacc
import concourse.tile as tile
from concourse import mybir
from concourse.bass_utils import run_bass_kernel_spmd

N_CORES = 8
B = 8192
D = 2048
C = 751
BS = B // N_CORES  # samples per core
P = 128
NT = BS // P       # sample tiles per core (8)
KDR = NT // 2      # fp8 DoubleRow k-tiles (256 samples each)
NCH = 4            # 512-col feature chunks (one PSUM bank each)
CH = D // NCH      # 512
FP8 = mybir.dt.float8e4
CLIP_LO = 1e-12
MULT = mybir.AluOpType.mult

# Default geometry matches the reference's seed-0 data (max label span 99).
DEF_CR = 99
DEF_MT = 1

# Fast-path output columns: 0 gram diag (sum x^2, all tiles), 1-6 per-bank
# cross-term drains, 7 cn2 (||c||^2 per local class).
F_GRAM, F_STT0, F_CN, F_OUTW = 0, 1, 7, 8

# Column groups: one PSUM bank each; the tail groups are narrow so the
# final drain chain after the last DMA byte is short.
F_GROUPS = [(0, 512), (512, 1024), (1024, 1536), (1536, 1792), (1792, 2048)]

_NC_CACHE = {}


def build_nc(cr=DEF_CR, mt=DEF_MT):
    if mt == 1:
        return _build_fast(cr)
    return _build_generic(cr, mt)


def _build_fast(cr):
    """Optimized single-class-tile program (label span <= 128).

    x streams column-group-major: for each 512/256-col group, the 4
    DoubleRow k-pairs arrive back to back, their fp8 copies feed the S
    matmuls for that group's PSUM bank plus the gram matmuls, and the bank
    drains immediately -- so drains pipeline with the DMA stream instead of
    stacking up after it. Sum x^2 comes entirely from the gram bank's
    diagonal (fp8: ~1e-3 relative bias, far inside the 2e-2 gate).
    """
    nc = bacc.Bacc("TRN2", target_bir_lowering=False)
    x = nc.dram_tensor("x", [BS, D], mybir.dt.float32, kind="ExternalInput")
    labels = nc.dram_tensor("labels", [P, NT], mybir.dt.int32, kind="ExternalInput")
    centers = nc.dram_tensor("centers", [cr, D], mybir.dt.float32, kind="ExternalInput")
    out = nc.dram_tensor("partial", [P, F_OUTW], mybir.dt.float32, kind="ExternalOutput")

    x_r = x[:].rearrange("(t p) d -> p t d", p=P)  # x_r[p, t, :] = x[t*128+p]

    with tile.TileContext(nc) as tc, ExitStack() as ctx:
        xp = ctx.enter_context(tc.tile_pool(name="xp", bufs=6))
        sqp = ctx.enter_context(tc.tile_pool(name="sqp", bufs=1))
        stp = ctx.enter_context(tc.tile_pool(name="stp", bufs=2))
        perm = ctx.enter_context(tc.tile_pool(name="perm", bufs=1))
        psp = ctx.enter_context(tc.tile_pool(name="psp", bufs=1, space="PSUM"))

        # labels + centers ride the ACT HWDGE ring; x owns the SP ring
        lab = perm.tile([P, NT], mybir.dt.int32)
        nc.scalar.dma_start(out=lab[:], in_=labels[:])
        ct = perm.tile([P, D], mybir.dt.float32)
        if cr < P:
            # pad rows must be finite zeros; memset start must be 32-aligned
            nc.gpsimd.memset(ct[(cr // 32) * 32 :, :], 0.0)
        nc.scalar.dma_start(out=ct[:cr, :], in_=centers[:, :])

        iota_i = perm.tile([P, P], mybir.dt.int32)
        nc.gpsimd.iota(iota_i[:], pattern=[[1, P]], base=0, channel_multiplier=0)
        pidx_i = perm.tile([P, 1], mybir.dt.int32)
        nc.gpsimd.iota(pidx_i[:], pattern=[[0, 1]], base=0, channel_multiplier=1)
        iota_f = perm.tile([P, P], mybir.dt.float32)
        nc.vector.tensor_copy(out=iota_f[:], in_=iota_i[:])
        pidx_f = perm.tile([P, 1], mybir.dt.float32)
        nc.vector.tensor_copy(out=pidx_f[:], in_=pidx_i[:])
        lab_f = perm.tile([P, NT], mybir.dt.float32)
        nc.vector.tensor_copy(out=lab_f[:], in_=lab[:])
        # eye_f[p, j] = (j == p), mask for the gram-diagonal extraction
        eye_f = perm.tile([P, P], mybir.dt.float32)
        nc.vector.tensor_scalar(
            out=eye_f[:], in0=iota_f[:], scalar1=pidx_f[:],
            scalar2=None, op0=mybir.AluOpType.is_equal,
        )

        out_sb = perm.tile([P, F_OUTW], mybir.dt.float32)
        nc.vector.memset(out_sb[:], 0.0)

        # ||c||^2 early on ACT (its only compute)
        sqc = sqp.tile([P, D], mybir.dt.float32)
        nc.scalar.activation(
            out=sqc[:cr, :], in_=ct[:cr, :],
            func=mybir.ActivationFunctionType.Square,
            accum_out=out_sb[:cr, F_CN : F_CN + 1],
        )

        # fp8 DoubleRow-packed x and one-hot: [128, 2, *], j = sample tile 2k+j
        x8 = [perm.tile([P, 2, D], FP8, tag=f"x8_{k}", name=f"x8_{k}")
              for k in range(KDR)]
        oh8 = [perm.tile([P, 2, P], FP8, tag=f"oh8_{k}", name=f"oh8_{k}")
               for k in range(KDR)]
        for t in range(NT):
            k, j = divmod(t, 2)
            nc.vector.tensor_scalar(
                out=oh8[k][:, j, :], in0=iota_f[:], scalar1=lab_f[:, t : t + 1],
                scalar2=None, op0=mybir.AluOpType.is_equal,
            )

        # PSUM: 5 S banks (one per column group) + 1 gram bank
        ps = [psp.tile([P, CH], mybir.dt.float32, tag=f"ps{g}", name=f"ps{g}")
              for g in range(len(F_GROUPS))]
        gram = psp.tile([P, CH], mybir.dt.float32, tag="gram")

        last_g = len(F_GROUPS) - 1

        def drain(g):
            # accum (-2 * S_g) . C -> col on DVE
            lo, hi = F_GROUPS[g]
            so = stp.tile([P, CH], mybir.dt.float32, tag="so", name="so")
            nc.vector.scalar_tensor_tensor(
                out=so[:, : hi - lo], in0=ps[g][:, : hi - lo], scalar=-2.0,
                in1=ct[:, lo:hi], op0=MULT, op1=MULT,
                accum_out=out_sb[:, F_STT0 + g : F_STT0 + g + 1],
            )

        for g, (lo, hi) in enumerate(F_GROUPS):
            w = hi - lo
            for k in range(KDR):
                # one DMA per (k-pair, group): both DoubleRow rows arrive
                # together (uniform staging keeps the scheduler's DMA order)
                stg = xp.tile([P, 2, w], mybir.dt.float32, tag="stg")
                nc.sync.dma_start(out=stg[:], in_=x_r[:, 2 * k : 2 * k + 2, lo:hi])
                nc.vector.tensor_copy(out=x8[k][:, :, lo:hi], in_=stg[:])
                nc.tensor.matmul(
                    out=ps[g][:, :w], lhsT=oh8[k][:, :, :],
                    rhs=x8[k][:, :, lo:hi],
                    start=(k == 0), stop=(k == KDR - 1),
                    perf_mode=mybir.MatmulPerfMode.DoubleRow,
                )
                for a in range(lo // P, -(-hi // P)):
                    blk = slice(a * P, (a + 1) * P)
                    first = (g == 0 and k == 0 and a == 0)
                    last = (g == last_g and k == KDR - 1 and blk.stop >= hi)
                    nc.tensor.matmul(
                        out=gram[:, :P],
                        lhsT=x8[k][:, :, blk], rhs=x8[k][:, :, blk],
                        start=first, stop=last,
                        perf_mode=mybir.MatmulPerfMode.DoubleRow,
                        skip_group_check=True,
                    )
                if k == 2 and g >= 1:
                    # previous group's drain, deferred so it never blocks
                    # this group's copies on DVE's in-order queue
                    drain(g - 1)
        drain(last_g)

        # gram diagonal -> col 0 (one eye-masked stt)
        dso = stp.tile([P, P], mybir.dt.float32, tag="dso", name="dso")
        nc.vector.scalar_tensor_tensor(
            out=dso[:], in0=gram[:, :P], scalar=1.0, in1=eye_f[:],
            op0=MULT, op1=MULT, accum_out=out_sb[:, F_GRAM : F_GRAM + 1],
        )

        nc.sync.dma_start(out=out[:], in_=out_sb[:])
    nc.compile()
    return nc


# ---------------------------------------------------------------------------
# Generic fallback for pathological label distributions (span > 128 classes).
# Correctness-first: squares all on ACT, centers in mt class tiles.
G_FULL_T = NT - 2


def _gcols(mt):
    stt0 = G_FULL_T + 2 * NCH
    cn0 = stt0 + mt * NCH
    outw = -(-(cn0 + mt) // 4) * 4
    return stt0, cn0, outw


def _build_generic(cr, mt):
    M = P * mt
    STT0, CN0, OUTW = _gcols(mt)
    nc = bacc.Bacc("TRN2", target_bir_lowering=False)
    x = nc.dram_tensor("x", [BS, D], mybir.dt.float32, kind="ExternalInput")
    labels = nc.dram_tensor("labels", [P, NT], mybir.dt.int32, kind="ExternalInput")
    centers = nc.dram_tensor("centers", [cr, D], mybir.dt.float32, kind="ExternalInput")
    out = nc.dram_tensor("partial", [P, OUTW], mybir.dt.float32, kind="ExternalOutput")

    x_r = x[:].rearrange("(t p) d -> p t d", p=P)

    with tile.TileContext(nc) as tc, ExitStack() as ctx:
        xp = ctx.enter_context(tc.tile_pool(name="xp", bufs=3))
        sqp = ctx.enter_context(tc.tile_pool(name="sqp", bufs=2))
        csq = ctx.enter_context(tc.tile_pool(name="csq", bufs=2))
        stp = ctx.enter_context(tc.tile_pool(name="stp", bufs=2))
        perm = ctx.enter_context(tc.tile_pool(name="perm", bufs=1))
        psp = ctx.enter_context(tc.tile_pool(name="psp", bufs=1, space="PSUM"))

        lab = perm.tile([P, NT], mybir.dt.int32)
        nc.scalar.dma_start(out=lab[:], in_=labels[:])

        cts = []
        for m in range(mt):
            rows = min(cr - m * P, P)
            ct = perm.tile([P, D], mybir.dt.float32, tag=f"ct{m}")
            if rows < P:
                nc.gpsimd.memset(ct[(rows // 32) * 32 :, :], 0.0)
            nc.scalar.dma_start(out=ct[:rows, :], in_=centers[m * P : m * P + rows, :])
            cts.append(ct)

        iota_i = perm.tile([P, M], mybir.dt.int32)
        nc.gpsimd.iota(iota_i[:], pattern=[[1, M]], base=0, channel_multiplier=0)
        iota_f = perm.tile([P, M], mybir.dt.float32)
        nc.vector.tensor_copy(out=iota_f[:], in_=iota_i[:])
        lab_f = perm.tile([P, NT], mybir.dt.float32)
        nc.vector.tensor_copy(out=lab_f[:], in_=lab[:])

        out_sb = perm.tile([P, OUTW], mybir.dt.float32)
        nc.vector.memset(out_sb[:], 0.0)

        x8 = [perm.tile([P, 2, D], FP8, tag=f"x8_{k}", name=f"x8_{k}")
              for k in range(KDR)]
        oh8 = [perm.tile([P, 2, M], FP8, tag=f"oh8_{k}", name=f"oh8_{k}")
               for k in range(KDR)]
        for t in range(NT):
            k, j = divmod(t, 2)
            nc.vector.tensor_scalar(
                out=oh8[k][:, j, :], in0=iota_f[:], scalar1=lab_f[:, t : t + 1],
                scalar2=None, op0=mybir.AluOpType.is_equal,
            )

        for t in range(G_FULL_T):
            k, j = divmod(t, 2)
            xt = xp.tile([P, D], mybir.dt.float32, tag="xt")
            nc.sync.dma_start(out=xt[:], in_=x_r[:, t, :])
            sq = sqp.tile([P, D], mybir.dt.float32, tag="sq")
            nc.scalar.activation(
                out=sq[:], in_=xt[:], func=mybir.ActivationFunctionType.Square,
                accum_out=out_sb[:, t : t + 1],
            )
            if t == 0:
                for m in range(mt):
                    rows = min(cr - m * P, P)
                    sqc = sqp.tile([P, D], mybir.dt.float32, tag="sq")
                    nc.scalar.activation(
                        out=sqc[:rows, :], in_=cts[m][:rows, :],
                        func=mybir.ActivationFunctionType.Square,
                        accum_out=out_sb[:rows, CN0 + m : CN0 + m + 1],
                    )
            nc.vector.tensor_copy(out=x8[k][:, j, :], in_=xt[:])

        xtail = [perm.tile([P, D], mybir.dt.float32, tag=f"xt{t}", name=f"xtail{t}")
                 for t in (0, 1)]
        for i, t in enumerate((G_FULL_T, G_FULL_T + 1)):
            for n in range(NCH):
                sl = slice(n * CH, (n + 1) * CH)
                nc.sync.dma_start(out=xtail[i][:, sl], in_=x_r[:, t, sl])

        for i in range(2):
            for n in range(NCH):
                sl = slice(n * CH, (n + 1) * CH)
                cq = csq.tile([P, CH], mybir.dt.float32, tag="cq")
                nc.scalar.activation(
                    out=cq[:], in_=xtail[i][:, sl],
                    func=mybir.ActivationFunctionType.Square,
                    accum_out=out_sb[:, G_FULL_T + i * NCH + n :
                                     G_FULL_T + i * NCH + n + 1],
                )

        for n in range(NCH):
            sl = slice(n * CH, (n + 1) * CH)
            nc.vector.tensor_copy(out=x8[KDR - 1][:, 0, sl], in_=xtail[0][:, sl])
            nc.vector.tensor_copy(out=x8[KDR - 1][:, 1, sl], in_=xtail[1][:, sl])
        for m in range(mt):
            pr = [psp.tile([P, CH], mybir.dt.float32, tag=f"ps{n}", name="pr")
                  for n in range(NCH)]
            for k in range(KDR):
                for n in range(NCH):
                    nc.tensor.matmul(
                        out=pr[n][:], lhsT=oh8[k][:, :, m * P : (m + 1) * P],
                        rhs=x8[k][:, :, n * CH : (n + 1) * CH],
                        start=(k == 0), stop=(k == KDR - 1),
                        perf_mode=mybir.MatmulPerfMode.DoubleRow,
                    )
            for n in range(NCH):
                so = stp.tile([P, CH], mybir.dt.float32, tag="so", name="so")
                nc.vector.scalar_tensor_tensor(
                    out=so[:], in0=pr[n][:], scalar=-2.0,
                    in1=cts[m][:, n * CH : (n + 1) * CH],
                    op0=MULT, op1=MULT,
                    accum_out=out_sb[:, STT0 + m * NCH + n : STT0 + m * NCH + n + 1],
                )

        nc.sync.dma_start(out=out[:], in_=out_sb[:])
    nc.compile()
    return nc


def _get_nc(cr, mt):
    key = (cr, mt)
    if key not in _NC_CACHE:
        _NC_CACHE[key] = build_nc(cr, mt)
    return _NC_CACHE[key]


def _shard(x, labels, centers):
    """Sort samples by label, split into 8 equal chunks, build per-core
    inputs with a local (relabeled) class window and its center rows."""
    order = np.argsort(labels, kind="stable")
    ls = labels[order]
    bases, spans = [], []
    for k in range(N_CORES):
        lo, hi = ls[k * BS], ls[(k + 1) * BS - 1]
        bases.append(int(lo))
        spans.append(int(hi - lo + 1))
    cr = max(spans)
    mt = -(-cr // P)
    in_maps, counts = [], []
    for k in range(N_CORES):
        idx = order[k * BS : (k + 1) * BS]
        xs = np.ascontiguousarray(x[idx])
        lloc = (ls[k * BS : (k + 1) * BS] - bases[k]).astype(np.int32)
        # lab[p, t] = lloc[t*P + p], matching the x tile layout
        lab = np.ascontiguousarray(lloc.reshape(NT, P).T)
        cw = np.zeros((cr, D), dtype=np.float32)
        cw[: spans[k]] = centers[bases[k] : bases[k] + spans[k]]
        in_maps.append({"x": xs, "labels": lab, "centers": cw})
        counts.append(np.bincount(lloc, minlength=mt * P).astype(np.float64))
    return in_maps, counts, cr, mt


def make_in_maps(x, labels, centers):
    return _shard(np.asarray(x), np.asarray(labels).astype(np.int64),
                  np.asarray(centers))[0]


def _combine(partials, counts, mt):
    if mt == 1:
        cn0, ncols = F_CN, F_CN
    else:
        _, cn0, _ = _gcols(mt)
        ncols = cn0
    total = 0.0
    for k, p in enumerate(partials):
        p64 = p.astype(np.float64)
        total += float(np.sum(p64[:, :ncols]))
        # n_c * ||c_c||^2: host label histogram x device-computed ||c||^2
        cn2 = p64[:, cn0 : cn0 + mt]  # class m*128 + partition
        total += float(np.sum(counts[k].reshape(mt, P).T * cn2))
    total += float(B) * float(C - 1) * CLIP_LO
    return np.array(total / B, dtype=np.float32)


def kernel(**inputs) -> np.ndarray:
    x = np.ascontiguousarray(np.asarray(inputs["x"], dtype=np.float32))
    labels = np.asarray(inputs["labels"]).astype(np.int64)
    centers = np.ascontiguousarray(np.asarray(inputs["centers"], dtype=np.float32))
    assert x.shape == (B, D) and labels.shape == (B,) and centers.shape == (C, D)

    in_maps, counts, cr, mt = _shard(x, labels, centers)
    nc = _get_nc(cr, mt)
    res = run_bass_kernel_spmd(nc, in_maps, core_ids=list(range(N_CORES)))
    return _combine([r["partial"] for r in res.results], counts, mt)


# revision 20
# speedup vs baseline: 1.0541x; 1.0056x over previous
"""CenterLoss Trainium2 kernel (label-bucketed data-parallel over 8 cores).

loss = sum(clip(distmat * onehot(labels), 1e-12, 1e12)) / B,
distmat[i,c] = ||x_i - centers_c||^2. Only the (i, labels_i) entries survive
the mask; the B*(C-1) masked entries contribute exactly 1e-12 each (added
analytically on host). For this distribution d_i ~ 4096, so the clip never
binds and the sum decomposes exactly:

  sum_i d_i = sum_i ||x_i||^2 + sum_c n_c ||c_c||^2 - 2 sum_c <s_c, c_c>

with s = onehot(labels)^T @ x (computed per core over its local samples).

Sharding is the key bandwidth optimization: samples are SORTED BY LABEL on
host and split into 8 equal chunks of 1024. Each chunk spans a contiguous
label range (<= 128 classes for uniform labels), so each core only loads the
center rows its samples reference: ~0.8 MB instead of the full 6.15 MB
replica. Per-core HBM traffic drops from 14.6 MB to ~9.2 MB; x itself
(8.4 MB/core, read once) is the roofline term. Labels are relabeled to the
local window; the per-class counts for the n_c ||c_c||^2 term are a host
histogram (as in the torch reference's bincount) dotted with the
device-computed ||c_c||^2 column.

Device pipeline per core (fast path, 128-class window):
  - x streams as 7 full [128, 2048] tiles + tile 7 in column chunks
    (512x3 + 256x2) so the end-of-stream dependency chain is short.
  - ACT: Square+accum of tiles 0-5 and of the centers window (early);
    fp32->fp8 copies of tile 7's chunks at the tail (DVE is drain-bound).
  - DVE: fp32->fp8 copies of tiles 0-6, one-hot build, PSUM drains
    scalar_tensor_tensor (-2*S).C with accum, and a masked-diagonal stt.
  - PE: fp8 DoubleRow matmuls accumulate S = onehot^T x in 4 PSUM banks.
    ||x||^2 of tiles 6+7 comes from 16 extra "gram" matmuls x8^T @ x8
    accumulated in a 5th PSUM bank: its diagonal is sum x^2 over those 512
    samples per feature-residue; one eye-masked stt extracts it. This keeps
    all late-arriving-square work off ACT/DVE's critical tail.

Per-core output is a [128, OUTW] block of raw partial columns; host combine
(f64) sums them, adds <histogram, cn2> and B*(C-1)*1e-12, divides by B.
"""

from contextlib import ExitStack

import numpy as np

import concourse.bacc as bacc
import concourse.tile as tile
from concourse import mybir
from concourse.bass_utils import run_bass_kernel_spmd

N_CORES = 8
B = 8192
D = 2048
C = 751
BS = B // N_CORES  # samples per core
P = 128
NT = BS // P       # sample tiles per core (8)
KDR = NT // 2      # fp8 DoubleRow k-tiles (256 samples each)
NCH = 4            # 512-col feature chunks (one PSUM bank each)
CH = D // NCH      # 512
FP8 = mybir.dt.float8e4
CLIP_LO = 1e-12
MULT = mybir.AluOpType.mult

# Default geometry matches the reference's seed-0 data (max label span 99).
DEF_CR = 99
DEF_MT = 1

# Fast-path output columns: 0 gram diag (sum x^2, all tiles), 1-6 per-bank
# cross-term drains, 7 cn2 (||c||^2 per local class).
F_GRAM, F_STT0, F_CN, F_OUTW = 0, 1, 7, 8

# Column groups: one PSUM bank each; the tail groups are narrow so the
# final drain chain after the last DMA byte is short.
F_GROUPS = [(0, 512), (512, 1024), (1024, 1536), (1536, 1792), (1792, 2048)]

_NC_CACHE = {}


def build_nc(cr=DEF_CR, mt=DEF_MT):
    if mt == 1:
        return _build_fast(cr)
    return _build_generic(cr, mt)


def _build_fast(cr):
    """Optimized single-class-tile program (label span <= 128).

    x streams column-group-major: for each 512/256-col group, the 4
    DoubleRow k-pairs arrive back to back, their fp8 copies feed the S
    matmuls for that group's PSUM bank plus the gram matmuls, and the bank
    drains immediately -- so drains pipeline with the DMA stream instead of
    stacking up after it. Sum x^2 comes entirely from the gram bank's
    diagonal (fp8: ~1e-3 relative bias, far inside the 2e-2 gate).
    """
    nc = bacc.Bacc("TRN2", target_bir_lowering=False)
    x = nc.dram_tensor("x", [BS, D], mybir.dt.float32, kind="ExternalInput")
    labels = nc.dram_tensor("labels", [P, NT], mybir.dt.int32, kind="ExternalInput")
    centers = nc.dram_tensor("centers", [cr, D], mybir.dt.float32, kind="ExternalInput")
    out = nc.dram_tensor("partial", [P, F_OUTW], mybir.dt.float32, kind="ExternalOutput")

    x_r = x[:].rearrange("(t p) d -> p t d", p=P)  # x_r[p, t, :] = x[t*128+p]

    with tile.TileContext(nc) as tc, ExitStack() as ctx:
        xp = ctx.enter_context(tc.tile_pool(name="xp", bufs=6))
        sqp = ctx.enter_context(tc.tile_pool(name="sqp", bufs=1))
        stp = ctx.enter_context(tc.tile_pool(name="stp", bufs=2))
        perm = ctx.enter_context(tc.tile_pool(name="perm", bufs=1))
        psp = ctx.enter_context(tc.tile_pool(name="psp", bufs=1, space="PSUM"))

        # labels + centers ride the ACT HWDGE ring; x owns the SP ring
        lab = perm.tile([P, NT], mybir.dt.int32)
        nc.scalar.dma_start(out=lab[:], in_=labels[:])
        ct = perm.tile([P, D], mybir.dt.float32)
        if cr < P:
            # pad rows must be finite zeros; memset start must be 32-aligned
            nc.gpsimd.memset(ct[(cr // 32) * 32 :, :], 0.0)
        nc.scalar.dma_start(out=ct[:cr, :], in_=centers[:, :])

        iota_i = perm.tile([P, P], mybir.dt.int32)
        nc.gpsimd.iota(iota_i[:], pattern=[[1, P]], base=0, channel_multiplier=0)
        pidx_i = perm.tile([P, 1], mybir.dt.int32)
        nc.gpsimd.iota(pidx_i[:], pattern=[[0, 1]], base=0, channel_multiplier=1)
        iota_f = perm.tile([P, P], mybir.dt.float32)
        nc.vector.tensor_copy(out=iota_f[:], in_=iota_i[:])
        pidx_f = perm.tile([P, 1], mybir.dt.float32)
        nc.vector.tensor_copy(out=pidx_f[:], in_=pidx_i[:])
        lab_f = perm.tile([P, NT], mybir.dt.float32)
        nc.vector.tensor_copy(out=lab_f[:], in_=lab[:])
        # eye_f[p, j] = (j == p), mask for the gram-diagonal extraction
        eye_f = perm.tile([P, P], mybir.dt.float32)
        nc.vector.tensor_scalar(
            out=eye_f[:], in0=iota_f[:], scalar1=pidx_f[:],
            scalar2=None, op0=mybir.AluOpType.is_equal,
        )

        out_sb = perm.tile([P, F_OUTW], mybir.dt.float32)
        nc.vector.memset(out_sb[:], 0.0)

        # ||c||^2 early on ACT (its only compute)
        sqc = sqp.tile([P, D], mybir.dt.float32)
        nc.scalar.activation(
            out=sqc[:cr, :], in_=ct[:cr, :],
            func=mybir.ActivationFunctionType.Square,
            accum_out=out_sb[:cr, F_CN : F_CN + 1],
        )

        # fp8 DoubleRow-packed x and one-hot: [128, 2, *], j = sample tile 2k+j
        x8 = [perm.tile([P, 2, D], FP8, tag=f"x8_{k}", name=f"x8_{k}")
              for k in range(KDR)]
        oh8 = [perm.tile([P, 2, P], FP8, tag=f"oh8_{k}", name=f"oh8_{k}")
               for k in range(KDR)]
        for t in range(NT):
            k, j = divmod(t, 2)
            nc.vector.tensor_scalar(
                out=oh8[k][:, j, :], in0=iota_f[:], scalar1=lab_f[:, t : t + 1],
                scalar2=None, op0=mybir.AluOpType.is_equal,
            )

        # PSUM: 5 S banks (one per column group) + 1 gram bank
        ps = [psp.tile([P, CH], mybir.dt.float32, tag=f"ps{g}", name=f"ps{g}")
              for g in range(len(F_GROUPS))]
        gram = psp.tile([P, CH], mybir.dt.float32, tag="gram")

        last_g = len(F_GROUPS) - 1

        def drain(g):
            # accum (-2 * S_g) . C -> col on DVE
            lo, hi = F_GROUPS[g]
            so = stp.tile([P, CH], mybir.dt.float32, tag="so", name="so")
            nc.vector.scalar_tensor_tensor(
                out=so[:, : hi - lo], in0=ps[g][:, : hi - lo], scalar=-2.0,
                in1=ct[:, lo:hi], op0=MULT, op1=MULT,
                accum_out=out_sb[:, F_STT0 + g : F_STT0 + g + 1],
            )

        for g, (lo, hi) in enumerate(F_GROUPS):
            w = hi - lo
            for k in range(KDR):
                # one DMA per (k-pair, group): both DoubleRow rows arrive
                # together (uniform staging keeps the scheduler's DMA order).
                # The very last pair streams its two rows separately so the
                # final copy on the critical tail is half as long.
                stg = xp.tile([P, 2, w], mybir.dt.float32, tag="stg")
                if g == last_g and k == KDR - 1:
                    for j in range(2):
                        nc.sync.dma_start(
                            out=stg[:, j, :], in_=x_r[:, 2 * k + j, lo:hi])
                        nc.vector.tensor_copy(
                            out=x8[k][:, j, lo:hi], in_=stg[:, j, :])
                else:
                    nc.sync.dma_start(out=stg[:], in_=x_r[:, 2 * k : 2 * k + 2, lo:hi])
                    nc.vector.tensor_copy(out=x8[k][:, :, lo:hi], in_=stg[:])
                nc.tensor.matmul(
                    out=ps[g][:, :w], lhsT=oh8[k][:, :, :],
                    rhs=x8[k][:, :, lo:hi],
                    start=(k == 0), stop=(k == KDR - 1),
                    perf_mode=mybir.MatmulPerfMode.DoubleRow,
                )
                for a in range(lo // P, -(-hi // P)):
                    blk = slice(a * P, (a + 1) * P)
                    first = (g == 0 and k == 0 and a == 0)
                    last = (g == last_g and k == KDR - 1 and blk.stop >= hi)
                    nc.tensor.matmul(
                        out=gram[:, :P],
                        lhsT=x8[k][:, :, blk], rhs=x8[k][:, :, blk],
                        start=first, stop=last,
                        perf_mode=mybir.MatmulPerfMode.DoubleRow,
                        skip_group_check=True,
                    )
                if k == 2 and g >= 1:
                    # previous group's drain, deferred so it never blocks
                    # this group's copies on DVE's in-order queue
                    drain(g - 1)
        drain(last_g)

        # gram diagonal -> col 0 (one eye-masked stt)
        dso = stp.tile([P, P], mybir.dt.float32, tag="dso", name="dso")
        nc.vector.scalar_tensor_tensor(
            out=dso[:], in0=gram[:, :P], scalar=1.0, in1=eye_f[:],
            op0=MULT, op1=MULT, accum_out=out_sb[:, F_GRAM : F_GRAM + 1],
        )

        nc.sync.dma_start(out=out[:], in_=out_sb[:])
    nc.compile()
    return nc


# ---------------------------------------------------------------------------
# Generic fallback for pathological label distributions (span > 128 classes).
# Correctness-first: squares all on ACT, centers in mt class tiles.
G_FULL_T = NT - 2


def _gcols(mt):
    stt0 = G_FULL_T + 2 * NCH
    cn0 = stt0 + mt * NCH
    outw = -(-(cn0 + mt) // 4) * 4
    return stt0, cn0, outw


def _build_generic(cr, mt):
    M = P * mt
    STT0, CN0, OUTW = _gcols(mt)
    nc = bacc.Bacc("TRN2", target_bir_lowering=False)
    x = nc.dram_tensor("x", [BS, D], mybir.dt.float32, kind="ExternalInput")
    labels = nc.dram_tensor("labels", [P, NT], mybir.dt.int32, kind="ExternalInput")
    centers = nc.dram_tensor("centers", [cr, D], mybir.dt.float32, kind="ExternalInput")
    out = nc.dram_tensor("partial", [P, OUTW], mybir.dt.float32, kind="ExternalOutput")

    x_r = x[:].rearrange("(t p) d -> p t d", p=P)

    with tile.TileContext(nc) as tc, ExitStack() as ctx:
        xp = ctx.enter_context(tc.tile_pool(name="xp", bufs=3))
        sqp = ctx.enter_context(tc.tile_pool(name="sqp", bufs=2))
        csq = ctx.enter_context(tc.tile_pool(name="csq", bufs=2))
        stp = ctx.enter_context(tc.tile_pool(name="stp", bufs=2))
        perm = ctx.enter_context(tc.tile_pool(name="perm", bufs=1))
        psp = ctx.enter_context(tc.tile_pool(name="psp", bufs=1, space="PSUM"))

        lab = perm.tile([P, NT], mybir.dt.int32)
        nc.scalar.dma_start(out=lab[:], in_=labels[:])

        cts = []
        for m in range(mt):
            rows = min(cr - m * P, P)
            ct = perm.tile([P, D], mybir.dt.float32, tag=f"ct{m}")
            if rows < P:
                nc.gpsimd.memset(ct[(rows // 32) * 32 :, :], 0.0)
            nc.scalar.dma_start(out=ct[:rows, :], in_=centers[m * P : m * P + rows, :])
            cts.append(ct)

        iota_i = perm.tile([P, M], mybir.dt.int32)
        nc.gpsimd.iota(iota_i[:], pattern=[[1, M]], base=0, channel_multiplier=0)
        iota_f = perm.tile([P, M], mybir.dt.float32)
        nc.vector.tensor_copy(out=iota_f[:], in_=iota_i[:])
        lab_f = perm.tile([P, NT], mybir.dt.float32)
        nc.vector.tensor_copy(out=lab_f[:], in_=lab[:])

        out_sb = perm.tile([P, OUTW], mybir.dt.float32)
        nc.vector.memset(out_sb[:], 0.0)

        x8 = [perm.tile([P, 2, D], FP8, tag=f"x8_{k}", name=f"x8_{k}")
              for k in range(KDR)]
        oh8 = [perm.tile([P, 2, M], FP8, tag=f"oh8_{k}", name=f"oh8_{k}")
               for k in range(KDR)]
        for t in range(NT):
            k, j = divmod(t, 2)
            nc.vector.tensor_scalar(
                out=oh8[k][:, j, :], in0=iota_f[:], scalar1=lab_f[:, t : t + 1],
                scalar2=None, op0=mybir.AluOpType.is_equal,
            )

        for t in range(G_FULL_T):
            k, j = divmod(t, 2)
            xt = xp.tile([P, D], mybir.dt.float32, tag="xt")
            nc.sync.dma_start(out=xt[:], in_=x_r[:, t, :])
            sq = sqp.tile([P, D], mybir.dt.float32, tag="sq")
            nc.scalar.activation(
                out=sq[:], in_=xt[:], func=mybir.ActivationFunctionType.Square,
                accum_out=out_sb[:, t : t + 1],
            )
            if t == 0:
                for m in range(mt):
                    rows = min(cr - m * P, P)
                    sqc = sqp.tile([P, D], mybir.dt.float32, tag="sq")
                    nc.scalar.activation(
                        out=sqc[:rows, :], in_=cts[m][:rows, :],
                        func=mybir.ActivationFunctionType.Square,
                        accum_out=out_sb[:rows, CN0 + m : CN0 + m + 1],
                    )
            nc.vector.tensor_copy(out=x8[k][:, j, :], in_=xt[:])

        xtail = [perm.tile([P, D], mybir.dt.float32, tag=f"xt{t}", name=f"xtail{t}")
                 for t in (0, 1)]
        for i, t in enumerate((G_FULL_T, G_FULL_T + 1)):
            for n in range(NCH):
                sl = slice(n * CH, (n + 1) * CH)
                nc.sync.dma_start(out=xtail[i][:, sl], in_=x_r[:, t, sl])

        for i in range(2):
            for n in range(NCH):
                sl = slice(n * CH, (n + 1) * CH)
                cq = csq.tile([P, CH], mybir.dt.float32, tag="cq")
                nc.scalar.activation(
                    out=cq[:], in_=xtail[i][:, sl],
                    func=mybir.ActivationFunctionType.Square,
                    accum_out=out_sb[:, G_FULL_T + i * NCH + n :
                                     G_FULL_T + i * NCH + n + 1],
                )

        for n in range(NCH):
            sl = slice(n * CH, (n + 1) * CH)
            nc.vector.tensor_copy(out=x8[KDR - 1][:, 0, sl], in_=xtail[0][:, sl])
            nc.vector.tensor_copy(out=x8[KDR - 1][:, 1, sl], in_=xtail[1][:, sl])
        for m in range(mt):
            pr = [psp.tile([P, CH], mybir.dt.float32, tag=f"ps{n}", name="pr")
                  for n in range(NCH)]
            for k in range(KDR):
                for n in range(NCH):
                    nc.tensor.matmul(
                        out=pr[n][:], lhsT=oh8[k][:, :, m * P : (m + 1) * P],
                        rhs=x8[k][:, :, n * CH : (n + 1) * CH],
                        start=(k == 0), stop=(k == KDR - 1),
                        perf_mode=mybir.MatmulPerfMode.DoubleRow,
                    )
            for n in range(NCH):
                so = stp.tile([P, CH], mybir.dt.float32, tag="so", name="so")
                nc.vector.scalar_tensor_tensor(
                    out=so[:], in0=pr[n][:], scalar=-2.0,
                    in1=cts[m][:, n * CH : (n + 1) * CH],
                    op0=MULT, op1=MULT,
                    accum_out=out_sb[:, STT0 + m * NCH + n : STT0 + m * NCH + n + 1],
                )

        nc.sync.dma_start(out=out[:], in_=out_sb[:])
    nc.compile()
    return nc


def _get_nc(cr, mt):
    key = (cr, mt)
    if key not in _NC_CACHE:
        _NC_CACHE[key] = build_nc(cr, mt)
    return _NC_CACHE[key]


def _shard(x, labels, centers):
    """Sort samples by label, split into 8 equal chunks, build per-core
    inputs with a local (relabeled) class window and its center rows."""
    order = np.argsort(labels, kind="stable")
    ls = labels[order]
    bases, spans = [], []
    for k in range(N_CORES):
        lo, hi = ls[k * BS], ls[(k + 1) * BS - 1]
        bases.append(int(lo))
        spans.append(int(hi - lo + 1))
    cr = max(spans)
    mt = -(-cr // P)
    in_maps, counts = [], []
    for k in range(N_CORES):
        idx = order[k * BS : (k + 1) * BS]
        xs = np.ascontiguousarray(x[idx])
        lloc = (ls[k * BS : (k + 1) * BS] - bases[k]).astype(np.int32)
        # lab[p, t] = lloc[t*P + p], matching the x tile layout
        lab = np.ascontiguousarray(lloc.reshape(NT, P).T)
        cw = np.zeros((cr, D), dtype=np.float32)
        cw[: spans[k]] = centers[bases[k] : bases[k] + spans[k]]
        in_maps.append({"x": xs, "labels": lab, "centers": cw})
        counts.append(np.bincount(lloc, minlength=mt * P).astype(np.float64))
    return in_maps, counts, cr, mt


def make_in_maps(x, labels, centers):
    return _shard(np.asarray(x), np.asarray(labels).astype(np.int64),
                  np.asarray(centers))[0]


def _combine(partials, counts, mt):
    if mt == 1:
        cn0, ncols = F_CN, F_CN
    else:
        _, cn0, _ = _gcols(mt)
        ncols = cn0
    total = 0.0
    for k, p in enumerate(partials):
        p64 = p.astype(np.float64)
        total += float(np.sum(p64[:, :ncols]))
        # n_c * ||c_c||^2: host label histogram x device-computed ||c||^2
        cn2 = p64[:, cn0 : cn0 + mt]  # class m*128 + partition
        total += float(np.sum(counts[k].reshape(mt, P).T * cn2))
    total += float(B) * float(C - 1) * CLIP_LO
    return np.array(total / B, dtype=np.float32)


def kernel(**inputs) -> np.ndarray:
    x = np.ascontiguousarray(np.asarray(inputs["x"], dtype=np.float32))
    labels = np.asarray(inputs["labels"]).astype(np.int64)
    centers = np.ascontiguousarray(np.asarray(inputs["centers"], dtype=np.float32))
    assert x.shape == (B, D) and labels.shape == (B,) and centers.shape == (C, D)

    in_maps, counts, cr, mt = _shard(x, labels, centers)
    nc = _get_nc(cr, mt)
    res = run_bass_kernel_spmd(nc, in_maps, core_ids=list(range(N_CORES)))
    return _combine([r["partial"] for r in res.results], counts, mt)
